# revision 11
# baseline (speedup 1.0000x reference)
"""Trainium2 Bass kernel for CropSplit (SipMask-style crop + quadrant split).

Reference computation, per output pixel (y, x, n):
    inside = point (x, y) lies in box rois[n] = (x1, y1, x2, y2)
    cell   = which of the 2x2 ROI sub-cells the pixel falls in
    out[y, x, n] = inside ? data[cell, y, x, n] : 0

Strategy:
  - Shard along W across the 8 cores (25 columns each). Each output pixel is
    independent, so any spatial shard works; W-sharding with an
    [h -> partitions, (w, n) -> free] tile layout makes every DMA row a
    large CONTIGUOUS DRAM block (w,n are the two innermost axes), which is
    what the DMA engines and HBM want. (H-sharding was measured at only
    ~16 GB/s per SDMA engine: 800B chunks with 160KB strides.)
  - The roi-derived masks are tiny (O(N*(H+W))), computed on host in
    float32 with bit-identical arithmetic to the reference, shipped as
    uint8:
        hx[w, n]  = (cx == 1)   quadrant column select
        nix[w, n] = NOT inside_x
        hy[h, n]  = (cy == 1)   quadrant row select
        niy[h, n] = NOT inside_y
  - Per tile, 5 predicated DVE ops implement select + mask:
        cp(d0, hx, d1)     d0 = hx ? d1 : d0     (x-blend, cy=0 pair)
        cp(d2, hx, d3)     d2 = hx ? d3 : d2     (x-blend, cy=1 pair)
        cp(d0, hy, d2)     d0 = hy ? d2 : d0     (y-blend -> 4-way select)
        cp(d0, nix, 0)     zero outside the box in x
        cp(d0, niy, 0)     zero outside the box in y
    Masks constant along w use 0-stride broadcast views ([h,n] tiles);
    masks constant along h (partitions) are materialized once per core via
    DMA partition-broadcast.
  - DMA issue is split across both HWDGE sequencers (Sync for loads,
    Scalar for stores/masks) to halve descriptor-generation serialization.
"""

import numpy as np

C = 2
CC = C * C
H = W = N = 200
NCORES = 8
WS = W // NCORES  # 25 columns per core

# (h0, ph, p_off): h rows [h0, h0+ph) live at partitions [p_off, p_off+ph).
# Chunk 2 (72 rows) is DMA'd into partitions 28..100 so its transfers are
# split evenly across both SDMA engine groups (partitions <64 / >=64).
# DVE ops always run on all 128 partitions (cost depends only on the free
# dim); the out-of-window partitions compute garbage that is never stored.
H_CHUNKS = [(0, 128, 0), (128, 72, 28)]
W_BLOCKS = [(0, 2), (2, 5), (7, 6), (13, 6), (19, 6)]
DATA_BUFS = 4

_cache: dict = {}


def _build_module():
    import concourse.bacc as bacc
    import concourse.mybir as mybir
    from concourse.tile import TileContext

    f32 = mybir.dt.float32
    u8 = mybir.dt.uint8

    nc = bacc.Bacc(trn_type="TRN2", debug=False, num_devices=NCORES)
    data = nc.dram_tensor("data", [CC, H, WS, N], f32, kind="ExternalInput")
    # x-masks pre-broadcast across partitions on host (plain contiguous load
    # is much faster than a DMA partition-broadcast): [p, (hx|nix), w, n]
    mxb = nc.dram_tensor("mxb", [128, 2, WS, N], u8, kind="ExternalInput")
    # y-masks packed: [p, (hy_c0|niy_c0|hy_c1|niy_c1), n] (chunk2 zero-padded)
    myb = nc.dram_tensor("myb", [128, 4, N], u8, kind="ExternalInput")
    out = nc.dram_tensor("out", [H, WS, N], f32, kind="ExternalOutput")

    with TileContext(nc) as tc:
        with (
            tc.tile_pool(name="masks", bufs=1) as mpool,
            tc.tile_pool(name="dpool", bufs=DATA_BUFS) as dpool,
        ):
            zeros = mpool.tile([128, 1], f32)
            nc.vector.memset(zeros[:], 0.0)

            # y-masks packed in one small load: [128, 4, N]
            ymask = mpool.tile([128, 4, N], u8)
            nc.scalar.dma_start(ymask[:], myb[:])
            hy_t = [ymask[:, 0, :], ymask[:, 2, :]]
            niy_t = [ymask[:, 1, :], ymask[:, 3, :]]

            # x-masks, already broadcast across partitions host-side
            xmask = mpool.tile([128, 2, WS, N], u8)
            nc.scalar.dma_start(xmask[:], mxb[:])
            hx_rep = xmask[:, 0]
            nix_rep = xmask[:, 1]

            for ci, (h0, ph, po) in enumerate(H_CHUNKS):
                sp = slice(po, po + ph)  # DMA partition window
                for w0, wb in W_BLOCKS:
                    # paired planes: d02 = (cell0, cell2), d13 = (cell1, cell3)
                    d02 = dpool.tile([128, 2, wb, N], f32, tag="d02")
                    d13 = dpool.tile([128, 2, wb, N], f32, tag="d13")
                    nc.sync.dma_start(
                        d02[sp],
                        data[0::2, h0 : h0 + ph, w0 : w0 + wb, :].transpose(
                            [1, 0, 2, 3]
                        ),
                    )
                    nc.sync.dma_start(
                        d13[sp],
                        data[1::2, h0 : h0 + ph, w0 : w0 + wb, :].transpose(
                            [1, 0, 2, 3]
                        ),
                    )
                    hxv2 = hx_rep[:, None, w0 : w0 + wb, :].broadcast_to(
                        (128, 2, wb, N)
                    )
                    hyv = hy_t[ci][:, None, :].broadcast_to((128, wb, N))
                    nixv = nix_rep[:, w0 : w0 + wb, :]
                    niyv = niy_t[ci][:, None, :].broadcast_to((128, wb, N))
                    zv = zeros[:, :, None].broadcast_to((128, wb, N))
                    # x-blend both cell rows in one op, then y-blend, then zero
                    nc.vector.copy_predicated(d02[:], hxv2, d13[:])
                    nc.vector.copy_predicated(d02[:, 0], hyv, d02[:, 1])
                    nc.vector.copy_predicated(d02[:, 0], nixv, zv)
                    nc.vector.copy_predicated(d02[:, 0], niyv, zv)
                    nc.scalar.dma_start(
                        out[h0 : h0 + ph, w0 : w0 + wb, :], d02[sp, 0]
                    )
    nc.finalize()
    return nc


def _get_module():
    if "nc" not in _cache:
        _cache["nc"] = _build_module()
    return _cache["nc"]


def _host_masks(rois):
    """Masks in f32 arithmetic bit-identical to the reference, as uint8."""
    r = np.asarray(rois, dtype=np.float32)
    x1, y1, x2, y2 = r[:, 0], r[:, 1], r[:, 2], r[:, 3]
    two = np.float32(2.0)
    one = np.float32(1.0)

    xs = np.arange(W, dtype=np.float32)[:, None]  # (W, 1)
    cw = np.maximum(x2 - x1, one)[None, :]  # (1, N)
    fx = np.floor(two * (xs - x1[None, :]) / cw)
    hx = (fx >= 1.0).astype(np.uint8)  # clip(floor, 0, 1) == 1
    nix = (~((xs >= x1[None, :]) & (xs <= x2[None, :]))).astype(np.uint8)

    ys = np.arange(H, dtype=np.float32)[:, None]  # (H, 1)
    ch = np.maximum(y2 - y1, one)[None, :]
    fy = np.floor(two * (ys - y1[None, :]) / ch)
    hy = (fy >= 1.0).astype(np.uint8)
    niy = (~((ys >= y1[None, :]) & (ys <= y2[None, :]))).astype(np.uint8)

    return hx, nix, hy, niy


def _run(data, rois, trace=False):
    from concourse.bass_utils import run_bass_kernel_spmd

    data = np.ascontiguousarray(np.asarray(data, dtype=np.float32))
    hx, nix, hy, niy = _host_masks(rois)

    # y-masks packed [128, 4, N]: chunk1 rows 0..127, chunk2 rows 128..199
    myb = np.zeros((128, 4, N), dtype=np.uint8)
    myb[:, 0] = hy[0:128]
    myb[:, 1] = niy[0:128]
    myb[28:100, 2] = hy[128:200]
    myb[28:100, 3] = niy[128:200]

    in_maps = []
    for i in range(NCORES):
        sl = slice(i * WS, (i + 1) * WS)
        mxb = np.ascontiguousarray(
            np.broadcast_to(
                np.stack([hx[sl, :], nix[sl, :]])[None], (128, 2, WS, N)
            )
        )
        in_maps.append(
            {
                "data": np.ascontiguousarray(data[:, :, sl, :]),
                "mxb": mxb,
                "myb": myb,
            }
        )

    nc = _get_module()
    last_err = None
    for _attempt in range(2):
        try:
            res = run_bass_kernel_spmd(
                nc, in_maps, core_ids=list(range(NCORES)), trace=trace
            )
            break
        except Exception as e:  # transient NRT device errors: retry once
            last_err = e
    else:
        raise last_err
    full = np.concatenate([r["out"] for r in res.results], axis=1)
    return np.asarray(full, dtype=np.float32), res


def kernel(data, rois):
    out, _ = _run(data, rois, trace=False)
    return out


# revision 26
# speedup vs baseline: 1.1614x; 1.1614x over previous
"""Trainium2 Bass kernel for CropSplit (SipMask-style crop + quadrant split).

Reference computation, per output pixel (y, x, n):
    inside = point (x, y) lies in box rois[n] = (x1, y1, x2, y2)
    cell   = which of the 2x2 ROI sub-cells the pixel falls in
    out[y, x, n] = inside ? data[cell, y, x, n] : 0

Strategy:
  - Shard along W across the 8 cores (25 columns each). Each output pixel is
    independent, so any spatial shard works; W-sharding with an
    [h -> partitions, (w, n) -> free] tile layout makes every DMA row a
    large CONTIGUOUS DRAM block (w,n are the two innermost axes), which is
    what the DMA engines and HBM want. (H-sharding was measured at only
    ~16 GB/s per SDMA engine: 800B chunks with 160KB strides.)
  - The roi-derived masks are computed on host in float32 with
    bit-identical arithmetic to the reference and shipped as uint8:
        hx[w, n]      = (cx == 1)            quadrant column select
        hy[h, n]      = (cy == 1)            quadrant row select
        nin[h, w, n]  = NOT inside(h, w, n)  outer-OR of the two box masks
    (hx is shipped pre-broadcast across partitions; a DMA
    partition-broadcast was measured much slower than a plain load.)
  - Per tile, the 4-way select + mask is 3 predicated DVE ops (cost of a
    DVE op depends only on the free-dim size, so the two x-blends run as
    one op over the plane-pair axis):
        cp(d02,    hx,  d13)   (d0 <- d1, d2 <- d3 where cx==1)
        cp(d02[0], hy,  d02[1])  (y-blend -> 4-way select)
        cp(d02[0], nin, 0)       (zero outside the box)
  - h-chunk 2 (rows 128..199) is DMA'd into partitions 28..100 so its
    transfers spread across both SDMA engine groups; DVE ops always run on
    all 128 partitions (free-dim-priced) and out-of-window partitions
    compute garbage that is never stored.
  - DMA issue is split across both HWDGE sequencers (Sync for data loads,
    Scalar for masks/stores); w-blocks are sized small-first for pipeline
    ramp, small-last for tail drain, 6-deep tile buffering in between.
"""

import numpy as np

C = 2
CC = C * C
H = W = N = 200
NCORES = 8
WS = W // NCORES  # 25 columns per core

# (h0, ph, p_off): h rows [h0, h0+ph) live at partitions [p_off, p_off+ph).
# Chunk 2 (72 rows) is DMA'd into partitions 28..100 so its transfers are
# split evenly across both SDMA engine groups (partitions <64 / >=64).
# DVE ops always run on all 128 partitions (cost depends only on the free
# dim); the out-of-window partitions compute garbage that is never stored.
H_CHUNKS = [(0, 128, 0), (128, 72, 28)]
W_BLOCKS = [(0, 3), (3, 6), (9, 6), (15, 6), (21, 4)]
DATA_BUFS = 6

_cache: dict = {}


def _build_module():
    import concourse.bacc as bacc
    import concourse.mybir as mybir
    from concourse.tile import TileContext

    f32 = mybir.dt.float32
    u8 = mybir.dt.uint8

    nc = bacc.Bacc(trn_type="TRN2", debug=False, num_devices=NCORES)
    data = nc.dram_tensor("data", [CC, H, WS, N], f32, kind="ExternalInput")
    # hx pre-broadcast across partitions on host: [p, w, n]
    mxb = nc.dram_tensor("mxb", [128, WS, N], u8, kind="ExternalInput")
    # per-pixel not-inside mask, packed per h-chunk: [p, chunk, w, n]
    ninb = nc.dram_tensor("ninb", [128, 2, WS, N], u8, kind="ExternalInput")
    # hy packed per h-chunk: [p, chunk, n]
    myb = nc.dram_tensor("myb", [128, 2, N], u8, kind="ExternalInput")
    out = nc.dram_tensor("out", [H, WS, N], f32, kind="ExternalOutput")

    with TileContext(nc) as tc:
        with (
            tc.tile_pool(name="masks", bufs=1) as mpool,
            tc.tile_pool(name="dpool", bufs=DATA_BUFS) as dpool,
        ):
            zeros = mpool.tile([128, 1], f32)
            nc.vector.memset(zeros[:], 0.0)

            # y-masks packed in one small load: [128, 2, N]
            ymask = mpool.tile([128, 2, N], u8)
            nc.scalar.dma_start(ymask[:], myb[:])
            hy_t = [ymask[:, 0, :], ymask[:, 1, :]]

            # x-masks, already broadcast across partitions host-side;
            # loaded per w-block so the first compute isn't gated on the
            # whole mask tensor.
            xm_blocks = []
            for bi, (w0, wb) in enumerate(W_BLOCKS):
                t_xm = mpool.tile([128, wb, N], u8, tag=f"xm{bi}")
                nc.scalar.dma_start(t_xm[:], mxb[:, w0 : w0 + wb, :])
                xm_blocks.append(t_xm)

            for ci, (h0, ph, po) in enumerate(H_CHUNKS):
                sp = slice(po, po + ph)  # DMA partition window
                for bi, (w0, wb) in enumerate(W_BLOCKS):
                    # paired planes: d02 = (cell0, cell2), d13 = (cell1, cell3)
                    d02 = dpool.tile([128, 2, wb, N], f32, tag="d02")
                    d13 = dpool.tile([128, 2, wb, N], f32, tag="d13")
                    nc.sync.dma_start(
                        d02[sp, 0], data[0, h0 : h0 + ph, w0 : w0 + wb, :]
                    )
                    nc.sync.dma_start(
                        d02[sp, 1], data[2, h0 : h0 + ph, w0 : w0 + wb, :]
                    )
                    nc.sync.dma_start(
                        d13[sp, 0], data[1, h0 : h0 + ph, w0 : w0 + wb, :]
                    )
                    nc.sync.dma_start(
                        d13[sp, 1], data[3, h0 : h0 + ph, w0 : w0 + wb, :]
                    )
                    t_nin = dpool.tile([128, wb, N], u8, tag="nin")
                    nc.scalar.dma_start(
                        t_nin[:], ninb[:, ci, w0 : w0 + wb, :]
                    )
                    hxv2 = xm_blocks[bi][:, None, :, :].broadcast_to(
                        (128, 2, wb, N)
                    )
                    hyv = hy_t[ci][:, None, :].broadcast_to((128, wb, N))
                    zv = zeros[:, :, None].broadcast_to((128, wb, N))
                    # x-blend both cell rows in one op, then y-blend, then zero
                    nc.vector.copy_predicated(d02[:], hxv2, d13[:])
                    nc.vector.copy_predicated(d02[:, 0], hyv, d02[:, 1])
                    nc.vector.copy_predicated(d02[:, 0], t_nin[:], zv)
                    nc.scalar.dma_start(
                        out[h0 : h0 + ph, w0 : w0 + wb, :], d02[sp, 0]
                    )
    nc.finalize()
    return nc


def _get_module():
    if "nc" not in _cache:
        _cache["nc"] = _build_module()
    return _cache["nc"]


def _host_masks(rois):
    """Masks in f32 arithmetic bit-identical to the reference, as uint8."""
    r = np.asarray(rois, dtype=np.float32)
    x1, y1, x2, y2 = r[:, 0], r[:, 1], r[:, 2], r[:, 3]
    two = np.float32(2.0)
    one = np.float32(1.0)

    xs = np.arange(W, dtype=np.float32)[:, None]  # (W, 1)
    cw = np.maximum(x2 - x1, one)[None, :]  # (1, N)
    fx = np.floor(two * (xs - x1[None, :]) / cw)
    hx = (fx >= 1.0).astype(np.uint8)  # clip(floor, 0, 1) == 1
    nix = (~((xs >= x1[None, :]) & (xs <= x2[None, :]))).astype(np.uint8)

    ys = np.arange(H, dtype=np.float32)[:, None]  # (H, 1)
    ch = np.maximum(y2 - y1, one)[None, :]
    fy = np.floor(two * (ys - y1[None, :]) / ch)
    hy = (fy >= 1.0).astype(np.uint8)
    niy = (~((ys >= y1[None, :]) & (ys <= y2[None, :]))).astype(np.uint8)

    return hx, nix, hy, niy


def _run(data, rois, trace=False):
    from concourse.bass_utils import run_bass_kernel_spmd

    data = np.ascontiguousarray(np.asarray(data, dtype=np.float32))
    hx, nix, hy, niy = _host_masks(rois)

    # hy packed [128, 2, N]: chunk1 rows 0..127, chunk2 rows 128..199@28..100
    myb = np.zeros((128, 2, N), dtype=np.uint8)
    myb[:, 0] = hy[0:128]
    myb[28:100, 1] = hy[128:200]

    in_maps = []
    for i in range(NCORES):
        sl = slice(i * WS, (i + 1) * WS)
        mxb = np.ascontiguousarray(
            np.broadcast_to(hx[sl, :][None], (128, WS, N))
        )
        # not-inside per pixel: nix(w,n) OR niy(h,n), packed per h-chunk
        nin = np.maximum(nix[sl, :][None, :, :], niy[:, None, :])  # (H, WS, N)
        ninb = np.zeros((128, 2, WS, N), dtype=np.uint8)
        ninb[:, 0] = nin[0:128]
        ninb[28:100, 1] = nin[128:200]
        in_maps.append(
            {
                "data": np.ascontiguousarray(data[:, :, sl, :]),
                "mxb": mxb,
                "ninb": np.ascontiguousarray(ninb),
                "myb": myb,
            }
        )

    nc = _get_module()
    last_err = None
    for _attempt in range(2):
        try:
            res = run_bass_kernel_spmd(
                nc, in_maps, core_ids=list(range(NCORES)), trace=trace
            )
            break
        except Exception as e:  # transient NRT device errors: retry once
            last_err = e
    else:
        raise last_err
    full = np.concatenate([r["out"] for r in res.results], axis=1)
    return np.asarray(full, dtype=np.float32), res


def kernel(data, rois):
    out, _ = _run(data, rois, trace=False)
    return out
